# revision 1
# baseline (speedup 1.0000x reference)
"""Trainium2 kernel for nn_Non_LinearGNN: fully on-device 8-core SPMD.

Device (per core, edge-sharded): DGE-gather node/neis features from an
AllGathered [featT | H0*scale] table, Xi MLP (16 fused 64-dim layers -> A),
Rou MLP (6 fused 32-dim layers + 5 fused 8-dim layers -> b), 2 message
passing iterations with on-device segment-sum (one-hot matmul into a
PSUM-resident [V,8] accumulator, window-uniform schedule), AllReduce for
H1, ReduceScatter for H2. Host: index/window binning, readout MLP + BN.

Edge layout: node windows of 128 ids; each (core, window) owns BPW blocks
of 128 edge slots; real edges with X_Node in window w fill that window's
slots, rest are padding (li sentinel 128 -> zero one-hot column).
"""

import os
import sys

import numpy as np

sys.path.insert(0, "/opt/trn_rl_repo")

import ml_dtypes

try:
    import jax
    jax.config.update("jax_compilation_cache_dir", "/tmp/jaxcache")
    jax.config.update("jax_persistent_cache_min_entry_size_bytes", -1)
    jax.config.update("jax_persistent_cache_min_compile_time_secs", 0)
except Exception:
    pass

import concourse.bacc as bacc
import concourse.mybir as mybir
import concourse.tile as tile
from concourse import bass, bass_utils
from concourse.masks import make_identity

BF16 = ml_dtypes.bfloat16

V = 50000
E = 400000
LN = 32
S = 8
ITER = 2
DEG = 8.0
MU = 0.8
D = LN + S
EPS = 1e-5
SCALE = MU / S / DEG
NCORES = 8

NW = 392                  # node windows of 128
VP = NW * 128             # 50176 (H tables, padded V)
VT = VP + 128             # 50304 combined-table rows (sentinel row space)
VSH = VP // NCORES        # 6272 H shard rows
FSH = VT // NCORES        # 6288 table shard rows

_CACHE = {}
LAST_RESULT = {}


def _build_nc(BPW, NW=NW, NCH=4, dbg=False, mp=True, fancy=True):
    VP = NW * 128             # padded V (H tables)
    VT = VP + 128             # combined-table rows (sentinel row space)
    VSH = VP // NCORES
    FSH = VT // NCORES
    NBLK = NW * BPW           # blocks of 128 edges per core
    NQ = NBLK // 4            # groups of 4 blocks
    QPC = NQ // NCH           # 4-groups per chunk
    BC = QPC * 4              # blocks per chunk
    CC2 = BC * 64             # 2-pack cols per chunk
    CC4 = BC * 32             # 4-pack cols per chunk
    assert NQ % NCH == 0

    nc = bacc.Bacc("TRN2", target_bir_lowering=False, debug=False)
    dt = mybir.dt
    AF = mybir.ActivationFunctionType
    OP = mybir.AluOpType

    ft_d = nc.declare_dram_parameter("ftsh", [FSH, 40], dt.bfloat16, isOutput=False)
    li8_d = nc.declare_dram_parameter("li8", [128, NBLK], dt.uint8, isOutput=False)
    ixe_d = nc.declare_dram_parameter("ixe", [128, NBLK], dt.uint16, isOutput=False)
    wxr_d = nc.declare_dram_parameter("wxr", [128, 544], dt.bfloat16, isOutput=False)
    bx_d = nc.declare_dram_parameter("bx", [128, 8], dt.float32, isOutput=False)
    hsh_d = nc.declare_dram_parameter("hsh", [VSH, 8], dt.bfloat16, isOutput=True)
    if dbg:
        dxp_d = nc.declare_dram_parameter("dxp", [128, NCH * CC2], dt.bfloat16,
                                          isOutput=True)
        dA_d = nc.declare_dram_parameter("dA", [128, NBLK * 64], dt.bfloat16,
                                         isOutput=True)
        dbe_d = nc.declare_dram_parameter("dbe", [128, NBLK * 8], dt.float32,
                                          isOutput=True)
        dh0_d = nc.declare_dram_parameter("dh0", [128, NBLK * 8], dt.bfloat16,
                                          isOutput=True)
        dh1_d = nc.declare_dram_parameter("dh1", [VP, 8], dt.bfloat16,
                                          isOutput=True)
        dmsg_d = nc.declare_dram_parameter("dmsg", [128, NBLK * 8], dt.bfloat16,
                                           isOutput=True)
        doh_d = nc.declare_dram_parameter("doh", [128, 512], dt.bfloat16,
                                          isOutput=True)
        dhsb_d = nc.declare_dram_parameter("dhsb", [128, NW * 8], dt.bfloat16,
                                           isOutput=True)
        dh1d_d = nc.declare_dram_parameter("dh1d", [VP, 8], dt.bfloat16,
                                           isOutput=True)

    grp = [list(range(NCORES))]

    with tile.TileContext(nc) as tc:
        with (
            tc.tile_pool(name="res", bufs=1) as res,
            tc.tile_pool(name="dram", bufs=1, space="DRAM") as dram,
        ):
            # ---- resident tiles
            wxr = res.tile([128, 544], dt.bfloat16)
            bx = res.tile([128, 8], dt.float32)
            li8 = res.tile([128, NBLK], dt.uint8)
            ixe16 = res.tile([128, NBLK], dt.uint16)
            ixe = res.tile([128, NBLK], dt.int32)
            li = res.tile([128, NBLK], dt.int32)
            ident = res.tile([128, 128], dt.bfloat16)
            iota_oh = res.tile([128, 128], dt.int32)
            b_e = res.tile([128, NBLK * 8], dt.float32)
            h0e = res.tile([128, NBLK * 8], dt.bfloat16)
            hsb2 = res.tile([128, NW * 8], dt.bfloat16)

            nc.sync.dma_start(wxr[:], wxr_d[:, :])
            nc.sync.dma_start(bx[:], bx_d[:, :])
            nc.sync.dma_start(li8[:], li8_d[:, :])
            nc.sync.dma_start(ixe16[:], ixe_d[:, :])
            nc.vector.tensor_copy(li[:], li8[:])
            nc.vector.tensor_copy(ixe[:], ixe16[:])
            make_identity(nc, ident[:])
            nc.gpsimd.iota(iota_oh[:], pattern=[[1, 128]], base=0,
                           channel_multiplier=0)

            # ---- AllGather the combined [featT | H0*scale] table
            ft_b = dram.tile([FSH, 40], dt.bfloat16)
            ftF = dram.tile([VT, 40], dt.bfloat16)
            nc.sync.dma_start(ft_b[:], ft_d[:, :])
            nc.gpsimd.collective_compute(
                "AllGather", OP.bypass, replica_groups=grp,
                ins=[ft_b[:].opt()], outs=[ftF[:].opt()])

            A_dram = dram.tile([128, NBLK * 64], dt.bfloat16)
            H1d = dram.tile([VP, 8], dt.bfloat16)
            H1F = dram.tile([VP, 8], dt.bfloat16)
            H2d = dram.tile([VP, 8], dt.bfloat16)
            rsb = dram.tile([VSH, 8], dt.bfloat16)

            # Xi layer schedule: (wslot, bias_col, residual, out)
            xi_sched = []
            for i in range(5):
                xi_sched.append((0, 0, "xp", "h"))
            xi_sched.append((1, 1, None, "f2"))
            for i in range(5):
                xi_sched.append((2, 2, None, "h"))
                xi_sched.append((3, 3, "f2", "h"))

            # ================= phase 1+2: chunked gather + MLPs =================
            for ch in range(NCH):
                with (
                    tc.tile_pool(name=f"chs{ch}", bufs=1) as chs,
                    tc.tile_pool(name=f"gp{ch}", bufs=4) as gp,
                    tc.tile_pool(name=f"pst{ch}", bufs=2, space="PSUM") as pst,
                    tc.tile_pool(name=f"psm{ch}", bufs=1, space="PSUM") as psm,
                ):
                    xp = chs.tile([128, CC2], dt.bfloat16, tag="xp")
                    ht = chs.tile([128, CC2], dt.bfloat16, tag="ht")
                    f2t = chs.tile([128, CC2], dt.bfloat16, tag="f2")
                    xr = chs.tile([128, CC4], dt.bfloat16, tag="xr")
                    rh = chs.tile([128, CC4], dt.bfloat16, tag="rh")

                    # ---- gathers + transposes into 2-pack / 4-pack layouts
                    for q in range(QPC):
                        g = ch * QPC + q       # global 4-group
                        gE = gp.tile([128, 160], dt.bfloat16, tag="gE")
                        for k in range(4):
                            b = 4 * g + k
                            nc.gpsimd.indirect_dma_start(
                                out=gE[:, 40 * k:40 * k + 40], out_offset=None,
                                in_=ftF[:],
                                in_offset=bass.IndirectOffsetOnAxis(
                                    ap=ixe[:, b:b + 1], axis=0))
                            # h0[neis] slice into resident buffer
                            nc.vector.tensor_copy(
                                h0e[:, 8 * b:8 * b + 8],
                                gE[:, 40 * k + 32:40 * k + 40])
                        # compact neis feats then transpose
                        gEc = gp.tile([128, 128], dt.bfloat16, tag="gEc")
                        for k in range(4):
                            nc.scalar.activation(
                                gEc[:, 32 * k:32 * k + 32],
                                gE[:, 40 * k:40 * k + 32], AF.Copy)
                        tpE = pst.tile([128, 128], dt.bfloat16, tag="tp")
                        nc.tensor.transpose(tpE[:], gEc[:], ident[:])
                        # node side: one-hot matmul gather (no DGE)
                        wg0 = (4 * g) // BPW
                        nwin = (4 * g + 3) // BPW - wg0 + 1
                        ftw = gp.tile([128, 40 * nwin], dt.bfloat16, tag="ftw")
                        for wl in range(nwin):
                            r0 = 128 * (wg0 + wl)
                            nc.sync.dma_start(ftw[:, 40 * wl:40 * wl + 40],
                                              ftF[r0:r0 + 128, :])
                        oh4n = gp.tile([128, 512], dt.bfloat16, tag="ohn")
                        nc.vector.tensor_tensor(
                            out=oh4n[:].rearrange("p (i v) -> p i v", i=4),
                            in0=li[:, 4 * g:4 * g + 4]
                                .rearrange("p (i o) -> p i o", i=4)
                                .to_broadcast([128, 4, 128]),
                            in1=iota_oh[:].rearrange("p (o v) -> p o v", o=1)
                                .to_broadcast([128, 4, 128]),
                            op=OP.is_equal)
                        for k in range(4):
                            b = 4 * g + k
                            wl = b // BPW - wg0
                            tpO = pst.tile([128, 128], dt.bfloat16, tag="tp")
                            nc.tensor.transpose(
                                tpO[:], oh4n[:, 128 * k:128 * k + 128],
                                ident[:])
                            ohT = gp.tile([128, 128], dt.bfloat16, tag="ohT")
                            nc.scalar.activation(ohT[:], tpO[:], AF.Copy)
                            xnp = pst.tile([128, 128], dt.float32, tag="xn")
                            nc.tensor.matmul(
                                xnp[0:32, :],
                                ftw[:, 40 * wl:40 * wl + 32], ohT[:],
                                start=True, stop=True)
                            pr = 2 * q + k // 2
                            lo = 64 * (k % 2)
                            nc.scalar.activation(
                                xp[lo:lo + 32, 128 * pr:128 * pr + 128],
                                xnp[0:32, :], AF.Copy)
                            nc.vector.tensor_copy(
                                xr[32 * k:32 * k + 32,
                                   128 * q:128 * q + 128],
                                xnp[0:32, :])
                            nc.scalar.activation(
                                xp[lo + 32:lo + 64, 128 * pr:128 * pr + 128],
                                tpE[32 * k:32 * k + 32, :], AF.Copy)
                    if dbg:
                        nc.sync.dma_start(dxp_d[:, ch * CC2:(ch + 1) * CC2],
                                          xp[:])

                    # ---- Xi MLP on chunk (2-pack, quadrants)
                    def wslot(i):
                        return wxr[:, 64 * i:64 * (i + 1)]

                    ABLK = 2048
                    nab = (CC2 + ABLK - 1) // ABLK
                    h = xp
                    for (slot, bcol, rk, outk) in xi_sched:
                        w = wslot(slot)
                        bias = bx[:, bcol:bcol + 1]
                        hn = f2t if outk == "f2" else ht
                        rest = {"xp": xp, "f2": f2t, None: None}[rk]
                        for bi in range(nab):
                            c0 = bi * ABLK
                            cw = min(ABLK, CC2 - c0)
                            ps = psm.tile([128, ABLK], dt.float32, tag="psA")
                            for s0 in range(0, cw, 512):
                                sw = min(512, cw - s0)
                                cs = slice(c0 + s0, c0 + s0 + sw)
                                qs = slice(s0, s0 + sw)
                                nc.tensor.matmul(
                                    ps[0:64, qs], w[0:64, :], h[0:64, cs],
                                    start=True, stop=(rest is None),
                                    tile_position=(0, 0))
                                if rest is not None:
                                    nc.tensor.matmul(
                                        ps[0:64, qs], wxr[0:64, 256:320],
                                        rest[0:64, cs], start=False, stop=True,
                                        tile_position=(0, 0))
                                nc.tensor.matmul(
                                    ps[64:128, qs], w[64:128, :], h[64:128, cs],
                                    start=True, stop=(rest is None),
                                    tile_position=(64, 64))
                                if rest is not None:
                                    nc.tensor.matmul(
                                        ps[64:128, qs], wxr[64:128, 256:320],
                                        rest[64:128, cs], start=False, stop=True,
                                        tile_position=(64, 64))
                            bs = slice(c0, c0 + cw)
                            nc.scalar.activation(hn[:, bs], ps[:, 0:cw],
                                                 AF.Prelu, bias=bias,
                                                 scale=1.0, alpha=0.25)
                        h = hn
                    a_fm = h    # [128, CC2] bf16 (2-pack A, unscaled)

                    # ---- A pair-transposes -> edge-major, staged, one DMA out
                    ast = chs.tile([128, CC2], dt.bfloat16, tag="xp")
                    for pr in range(BC // 2):
                        tpA = pst.tile([128, 128], dt.bfloat16, tag="tp")
                        nc.tensor.transpose(
                            tpA[:], a_fm[:, 128 * pr:128 * pr + 128], ident[:])
                        nc.scalar.activation(ast[:, 128 * pr:128 * pr + 128],
                                             tpA[:], AF.Copy)
                    nc.sync.dma_start(A_dram[:, ch * CC2:(ch + 1) * CC2], ast[:])
                    if dbg:
                        nc.sync.dma_start(dA_d[:, ch * CC2:(ch + 1) * CC2],
                                          ast[:])

                    # ---- Rou on chunk (4-pack)
                    rf2 = chs.tile([128, CC4], dt.bfloat16, tag="xp")
                    rb1 = chs.tile([128, CC4], dt.bfloat16, tag="ht")
                    hr = xr
                    RB = 512
                    nrb = (CC4 + RB - 1) // RB
                    for lay in range(6):
                        rn = rf2 if lay == 5 else rh
                        for bi in range(nrb):
                            c0 = bi * RB
                            cw = min(RB, CC4 - c0)
                            cs = slice(c0, c0 + cw)
                            ps = psm.tile([128, RB], dt.float32, tag="psA")
                            if lay == 5:
                                nc.tensor.matmul(ps[0:32, 0:cw], wxr[:, 448:480],
                                                 hr[:, cs], start=True,
                                                 stop=True)
                                nc.scalar.activation(rn[0:32, cs],
                                                     ps[0:32, 0:cw],
                                                     AF.Prelu,
                                                     bias=bx[0:32, 5:6],
                                                     scale=1.0, alpha=0.25)
                            else:
                                nc.tensor.matmul(ps[:, 0:cw], wxr[:, 320:448],
                                                 hr[:, cs], start=True,
                                                 stop=False)
                                nc.tensor.matmul(ps[0:64, 0:cw],
                                                 wxr[0:64, 256:320],
                                                 xr[0:64, cs], start=False,
                                                 stop=True, tile_position=(0, 0))
                                nc.tensor.matmul(ps[64:128, 0:cw],
                                                 wxr[64:128, 256:320],
                                                 xr[64:128, cs], start=False,
                                                 stop=True,
                                                 tile_position=(64, 64))
                                nc.scalar.activation(rn[:, cs], ps[:, 0:cw],
                                                     AF.Prelu,
                                                     bias=bx[:, 4:5],
                                                     scale=1.0, alpha=0.25)
                        hr = rn
                    # 5 r3a layers on lanes 0:32 (4-pack-8), residual rf2
                    cur = rf2
                    for lay in range(5):
                        for bi in range(nrb):
                            c0 = bi * RB
                            cw = min(RB, CC4 - c0)
                            cs = slice(c0, c0 + cw)
                            ps = psm.tile([128, RB], dt.float32, tag="psA")
                            nc.tensor.matmul(ps[0:32, 0:cw], wxr[0:32, 480:512],
                                             cur[0:32, cs], start=True,
                                             stop=False)
                            nc.tensor.matmul(ps[0:32, 0:cw], wxr[0:32, 512:544],
                                             rf2[0:32, cs], start=False,
                                             stop=True)
                            nc.scalar.activation(rb1[0:32, cs], ps[0:32, 0:cw],
                                                 AF.Prelu, bias=bx[0:32, 6:7],
                                                 scale=1.0, alpha=0.25)
                        cur = rb1
                    # ---- b transposes -> b_e edge-major (fp32)
                    for q in range(QPC):
                        g = ch * QPC + q
                        tpB = pst.tile([128, 128], dt.bfloat16, tag="tp")
                        nc.tensor.transpose(
                            tpB[:, 0:32], cur[0:32, 128 * q:128 * q + 128],
                            ident[0:32, 0:32])
                        nc.scalar.activation(b_e[:, 32 * g:32 * g + 32],
                                             tpB[:, 0:32], AF.Copy)

            if dbg:
                nc.sync.dma_start(dbe_d[:, :], b_e[:])
                nc.sync.dma_start(dh0_d[:, :], h0e[:])

            # ================= message passing =================
            for it in range(ITER if mp else 0):
                with (
                    tc.tile_pool(name=f"psh{it}", bufs=1, space="PSUM") as psh,
                    tc.tile_pool(name=f"ap{it}", bufs=3) as apl,
                ):
                    Hp = psh.tile([128, NW * 8], dt.float32)
                    if dbg and it == 0:
                        msgbuf = res.tile([128, NBLK * 8], dt.bfloat16,
                                          name=f"msgbuf{it}")
                    for g in range(NQ):
                        Ast = apl.tile([128, 256], dt.bfloat16, tag="Ast")
                        nc.sync.dma_start(Ast[:],
                                          A_dram[:, 256 * g:256 * (g + 1)])
                        if it == 0:
                            he = h0e[:, 32 * g:32 * g + 32]
                        else:
                            het = apl.tile([128, 32], dt.bfloat16, tag="het")
                            for k in range(4):
                                b = 4 * g + k
                                nc.gpsimd.indirect_dma_start(
                                    out=het[:, 8 * k:8 * k + 8],
                                    out_offset=None, in_=H1F[:],
                                    in_offset=bass.IndirectOffsetOnAxis(
                                        ap=ixe[:, b:b + 1], axis=0))
                            he = het[:, :]
                        prod = apl.tile([128, 256], dt.float32, tag="prod")
                        nc.vector.tensor_tensor(
                            out=prod[:].rearrange("p (i u j) -> p i u j",
                                                  i=4, u=8),
                            in0=Ast[:].rearrange("p (i u j) -> p i u j",
                                                 i=4, u=8),
                            in1=he.rearrange("p (i o j) -> p i o j", i=4, o=1)
                                .to_broadcast([128, 4, 8, 8]),
                            op=OP.mult)
                        msum = apl.tile([128, 32], dt.float32, tag="msum")
                        nc.vector.tensor_reduce(
                            out=msum[:],
                            in_=prod[:].rearrange("p (i u j) -> p i u j",
                                                  i=4, u=8),
                            axis=mybir.AxisListType.X, op=OP.add)
                        msgb = apl.tile([128, 32], dt.bfloat16, tag="msgb")
                        nc.vector.tensor_tensor(
                            out=msgb[:], in0=msum[:],
                            in1=b_e[:, 32 * g:32 * g + 32], op=OP.add)
                        oh4 = apl.tile([128, 512], dt.bfloat16, tag="oh4")
                        nc.vector.tensor_tensor(
                            out=oh4[:].rearrange("p (i v) -> p i v", i=4),
                            in0=li[:, 4 * g:4 * g + 4]
                                .rearrange("p (i o) -> p i o", i=4)
                                .to_broadcast([128, 4, 128]),
                            in1=iota_oh[:].rearrange("p (o v) -> p o v", o=1)
                                .to_broadcast([128, 4, 128]),
                            op=OP.is_equal)
                        if dbg and it == 0:
                            nc.vector.tensor_copy(
                                msgbuf[:, 32 * g:32 * g + 32], msgb[:])
                            if g == 0:
                                nc.sync.dma_start(doh_d[:, :], oh4[:])
                        for k in range(4):
                            b = 4 * g + k
                            w = b // BPW
                            j = b % BPW
                            nc.tensor.matmul(
                                Hp[:, 8 * w:8 * w + 8],
                                oh4[:, 128 * k:128 * k + 128],
                                msgb[:, 8 * k:8 * k + 8],
                                start=(j == 0), stop=(j == BPW - 1))
                    # psum -> sbuf (scale on iter 0 for the next gather table)
                    sc = SCALE if it == 0 else 1.0
                    hout = hsb2
                    for c0 in range(0, NW * 8, 512):
                        cw = min(512, NW * 8 - c0)
                        nc.scalar.activation(hout[:, c0:c0 + cw],
                                             Hp[:, c0:c0 + cw], AF.Copy,
                                             scale=sc)
                    # sbuf [p, (w s)] -> dram [(w p) s]
                    hd = H1d if it == 0 else H2d
                    if fancy:
                        nc.sync.dma_start(
                            hd[:].rearrange("(w p) s -> w p s", p=128),
                            hsb2[:].rearrange("p (w s) -> w p s", w=NW))
                    else:
                        for w in range(NW):
                            nc.sync.dma_start(hd[128 * w:128 * w + 128, :],
                                              hout[:, 8 * w:8 * w + 8])
                    if it == 0:
                        if dbg:
                            nc.sync.dma_start(dmsg_d[:, :], msgbuf[:])
                            nc.sync.dma_start(dhsb_d[:, :], hsb2[:])
                            nc.sync.dma_start(dh1d_d[:, :], H1d[:])
                        nc.gpsimd.collective_compute(
                            "AllReduce", OP.add, replica_groups=grp,
                            ins=[H1d[:].opt()], outs=[H1F[:].opt()])
                        if dbg:
                            nc.sync.dma_start(dh1_d[:, :], H1F[:])
                    else:
                        nc.gpsimd.collective_compute(
                            "ReduceScatter", OP.add, replica_groups=grp,
                            ins=[H2d[:].opt()], outs=[rsb[:].opt()])
                        nc.sync.dma_start(hsh_d[:, :], rsb[:])
            if not mp:
                zt = res.tile([128, 8], dt.bfloat16)
                nc.vector.memset(zt[:], 0.0)
                nc.sync.dma_start(hsh_d[0:128, :], zt[:])

    nc.compile()
    return nc


def _prelu(x, a):
    return np.where(x >= 0, x, a * x)


def kernel(**inputs):
    X_Node = np.asarray(inputs["X_Node"]).astype(np.int64)
    X_Neis = np.asarray(inputs["X_Neis"]).astype(np.int64)
    fM = np.asarray(inputs["feature_Matrix"], dtype=np.float32)
    H0 = np.asarray(inputs["node_states"], dtype=np.float32)
    g = {k: np.asarray(v, dtype=np.float32) for k, v in inputs.items()
         if k not in ("X_Node", "X_Neis")}

    # ---- host: bin edges by (window, core)
    win = (X_Node >> 7).astype(np.int64)
    order = np.argsort(win, kind="stable")
    counts = np.bincount(win, minlength=NW)
    starts = np.zeros(NW + 1, np.int64)
    np.cumsum(counts, out=starts[1:])
    maxc = int(max(-(-counts[w] // NCORES) for w in range(NW)))
    BPW = max(2, -(-maxc // 128))
    NBLK = NW * BPW

    li8 = np.full((NCORES, 128, NBLK), 128, np.uint8)
    ixe = np.zeros((NCORES, 128, NBLK), np.uint16)
    wb = (np.arange(NBLK) // BPW) * 128
    for w in range(NW):
        ew = order[starts[w]:starts[w + 1]]
        cnt = len(ew)
        if cnt == 0:
            continue
        bounds = [(cnt * c) // NCORES for c in range(NCORES + 1)]
        for c in range(NCORES):
            sub = ew[bounds[c]:bounds[c + 1]]
            n = len(sub)
            if n == 0:
                continue
            sl = np.arange(n)
            bcol = BPW * w + sl // 128
            prow = sl % 128
            li8[c, prow, bcol] = X_Node[sub] - 128 * w
            ixe[c, prow, bcol] = X_Neis[sub]

    ftab = np.zeros((VT, 40), BF16)
    ftab[:V, 0:32] = fM.T.astype(BF16)
    ftab[:V, 32:40] = (H0 * SCALE).astype(BF16)

    # ---- weights
    I64 = np.eye(64, dtype=BF16)

    def bd4(w):
        out = np.zeros((4 * w.shape[0], 4 * w.shape[1]), dtype=BF16)
        for i in range(4):
            out[i * w.shape[0]:(i + 1) * w.shape[0],
                i * w.shape[1]:(i + 1) * w.shape[1]] = w.astype(BF16)
        return out

    wxr = np.zeros((128, 544), BF16)
    for i, wname in enumerate(["xi1w", "xi2w", "xi3w", "xi3aw"]):
        wt = g[wname].T.astype(BF16)
        wxr[0:64, 64 * i:64 * i + 64] = wt
        wxr[64:128, 64 * i:64 * i + 64] = wt
    wxr[0:64, 256:320] = I64
    wxr[64:128, 256:320] = I64
    wxr[:, 320:448] = bd4(g["r1w"].T)
    wxr[:, 448:480] = bd4(g["r2w"].T)
    wxr[0:32, 480:512] = bd4(g["r3aw"].T)
    wxr[0:32, 512:544] = np.eye(32, dtype=BF16)

    bxv = np.zeros((128, 8), np.float32)
    for i, bn in enumerate(["xi1b", "xi2b", "xi3b", "xi3ab"]):
        bxv[0:64, i] = g[bn]
        bxv[64:128, i] = g[bn]
    bxv[:, 4] = np.tile(g["r1b"], 4)
    bxv[0:32, 5] = np.tile(g["r2b"], 4)
    bxv[0:32, 6] = np.tile(g["r3ab"], 4)

    in_maps = []
    for c in range(NCORES):
        in_maps.append({
            "ftsh": ftab[FSH * c:FSH * (c + 1)],
            "li8": li8[c], "ixe": ixe[c],
            "wxr": wxr, "bx": bxv,
        })

    if ("nc", BPW) not in _CACHE:
        _CACHE[("nc", BPW)] = _build_nc(BPW, fancy=False)
    nc = _CACHE[("nc", BPW)]

    trace = bool(int(os.environ.get("KERNEL_TRACE", "0")))
    import time as _time
    try:
        t0 = _time.time()
        res = bass_utils.run_bass_kernel_spmd(
            nc, in_maps, core_ids=list(range(NCORES)), trace=trace)
    except ModuleNotFoundError:
        t0 = _time.time()
        res = bass_utils.run_bass_kernel_spmd(
            nc, in_maps, core_ids=list(range(NCORES)), trace=False)
    LAST_RESULT["run_wall_s"] = _time.time() - t0
    LAST_RESULT["exec_time_ns"] = res.exec_time_ns

    H2 = np.concatenate([res.results[c]["hsh"] for c in range(NCORES)],
                        axis=0)[:V].astype(np.float32)

    # ---- host readout (identical math to reference)
    out = np.concatenate([fM.T, H2], axis=1)
    o = out
    for _ in range(10):
        o = _prelu(o @ g["l1w"].T + g["l1b"], g["ga"])
        o = o @ g["l1aw"].T + g["l1ab"]
        o = _prelu(o + out, g["ga"])
        mean = o.mean(0)
        var = ((o - mean) ** 2).mean(0)
        o = (o - mean) / np.sqrt(var + EPS) * g["bn_g"] + g["bn_b"]
    o = o @ g["l2w"].T + g["l2b"]
    o2 = _prelu(o, g["ga"])
    o3 = o2
    for _ in range(10):
        o3 = _prelu(o3 @ g["l3w"].T + g["l3b"], g["ga"])
        o3 = o3 @ g["l3aw"].T + g["l3ab"]
        o3 = _prelu(o3 + o2, g["ga"])
    return np.concatenate([o3[:, 0], o3[:, 1]], axis=0).astype(np.float32)

